# revision 10
# baseline (speedup 1.0000x reference)
"""Multihead attention (B=2, S=2048, D=1024, H=16) on 8 TRN2 NeuronCores.

Sharding: core c -> batch b = c//4, head-group g = c%4 (4 heads, 256 features).
Each core computes q/k/v projections for its 256 features, attention for its
4 heads, and a row-parallel partial of the output projection. Host sums the
4 partials per batch (row-parallel TP unshard) and transposes back.

On-device layout (per core):
  xqT/xkT/xvT : [1024, 2048]  input.T (bf16)
  qT, kT      : [256, 2048]   projected, feature-major (scores need head-dim
                              on partitions)
  va          : [2048, 4*65]  v in [s, f] layout, per head 64 v-cols + ones col
                              (ones col makes the AV matmul emit the softmax
                              denominator as row 64 of its PSUM output)
  scoresT     : [128 j, 2048 i] PSUM per j-tile; exp(scale*x + mask_j) fused in
                              one ScalarE activation (mask is a per-partition bias)
  out partial : [1024, 2048]  (= partial_out.T, f32)

Schedule: everything cycles through ONE 2-slot PSUM pool ([128,2048] f32 slots,
4 banks each). q/k f-tile-0 projections run k-outer so they consume input DMA
as it streams; attention head 0 starts right after. The f-tile-1 projections,
the v projection (16 seq-tile units), and the previous head's AV (4 accumulator
quads packed 4-per-slot) are interleaved between score tiles of later phases so
the PE stays dense while ScalarE (exp = the critical 122us floor) never stalls.
"""

import math

import numpy as np

B, S, D, H = 2, 2048, 1024, 16
NCORES = 8
GH = 4                  # heads per core
HD = D // H             # 64
F = GH * HD             # 256 local features
SCALE = 1.0 / math.sqrt(HD)
NEG = np.float32(-9e9)

KT = D // 128           # 8 contraction tiles (projections)
FT = F // 128           # 2 local-feature tiles
ST = S // 128           # 16 sequence tiles
DT = D // 128           # 8 output-feature tiles

TRACE = False           # set by test harness; requires antenv.axon_hooks wired
LAST_EXEC_NS = None
LAST_RESULTS = None

_STATE = {}


def _build():
    import concourse.bacc as bacc
    import concourse.mybir as mybir
    from concourse import masks
    from concourse.tile import TileContext

    f32 = mybir.dt.float32
    bf16 = mybir.dt.bfloat16
    Exp = mybir.ActivationFunctionType.Exp

    nc = bacc.Bacc("TRN2", target_bir_lowering=False, debug=False,
                   num_devices=NCORES)

    xq_d = nc.declare_dram_parameter("xqT", [D, S], bf16, isOutput=False)
    xk_d = nc.declare_dram_parameter("xkT", [D, S], bf16, isOutput=False)
    xv_d = nc.declare_dram_parameter("xvT", [D, S], bf16, isOutput=False)
    wq_d = nc.declare_dram_parameter("wqT", [D, F], bf16, isOutput=False)
    wk_d = nc.declare_dram_parameter("wkT", [D, F], bf16, isOutput=False)
    wv_d = nc.declare_dram_parameter("wvT", [D, F], bf16, isOutput=False)
    wo_d = nc.declare_dram_parameter("woT", [F, D], bf16, isOutput=False)
    bq_d = nc.declare_dram_parameter("bq", [F], f32, isOutput=False)
    bk_d = nc.declare_dram_parameter("bk", [F], f32, isOutput=False)
    bv_d = nc.declare_dram_parameter("bv", [F], bf16, isOutput=False)
    bo_d = nc.declare_dram_parameter("bo", [D], f32, isOutput=False)
    mk_d = nc.declare_dram_parameter("maskf", [S], f32, isOutput=False)
    out_d = nc.declare_dram_parameter("outT", [D, S], f32, isOutput=True)

    with TileContext(nc) as tc:
        with tc.tile_pool(name="persist", bufs=1) as pp, \
             tc.tile_pool(name="xin", bufs=8) as xp, \
             tc.tile_pool(name="expp", bufs=26) as ep, \
             tc.tile_pool(name="ostage", bufs=2) as osp, \
             tc.tile_pool(name="small", bufs=8) as sp:

            def ptile(shape, dtype, name):
                return pp.tile(shape, dtype, name=name, tag=name)

            # ---- persistent SBUF tensors ----
            wq_sb = [ptile([128, F], bf16, f"wq{k}") for k in range(KT)]
            wk_sb = [ptile([128, F], bf16, f"wk{k}") for k in range(KT)]
            wv_sb = [ptile([128, F], bf16, f"wv{k}") for k in range(KT)]
            wo_sb = [ptile([128, D], bf16, f"wo{t}") for t in range(FT)]
            bq_sb = [ptile([128, 1], f32, f"bq{t}") for t in range(FT)]
            bk_sb = [ptile([128, 1], f32, f"bk{t}") for t in range(FT)]
            bo_sb = [ptile([128, 1], f32, f"bo{t}") for t in range(DT)]
            mk_sb = [ptile([128, 1], f32, f"mk{j}") for j in range(ST)]
            bv_sb = ptile([1, F], bf16, "bvrow")
            ones_sb = ptile([1, 128], bf16, "onesrow")
            ident = ptile([128, 128], bf16, "ident")
            qT_sb = [ptile([128, S], bf16, f"qT{t}") for t in range(FT)]
            kT_sb = [ptile([128, S], bf16, f"kT{t}") for t in range(FT)]
            va_sb = [ptile([128, GH * (HD + 1)], bf16, f"va{j}") for j in range(ST)]
            os_sb = [ptile([128, F], bf16, f"os{i}") for i in range(ST)]
            ot_sb = [ptile([128, S], bf16, f"ot{t}") for t in range(FT)]

            nc.vector.memset(ones_sb[:], 1.0)
            masks.make_identity(nc, ident[:])
            for j in range(ST):
                nc.vector.memset(va_sb[j][:], 1.0)

            # DMAs in consumption order: tiny constants, q/k weight+input
            # k-tile pairs (feeds the k-outer f0 projection as it streams),
            # then v, then the output-projection weights.
            for j in range(ST):
                nc.sync.dma_start(out=mk_sb[j][:],
                                  in_=mk_d[j * 128:(j + 1) * 128].unsqueeze(1))
            for t in range(FT):
                nc.sync.dma_start(out=bq_sb[t][:],
                                  in_=bq_d[t * 128:(t + 1) * 128].unsqueeze(1))
                nc.sync.dma_start(out=bk_sb[t][:],
                                  in_=bk_d[t * 128:(t + 1) * 128].unsqueeze(1))
            for t in range(DT):
                nc.sync.dma_start(out=bo_sb[t][:],
                                  in_=bo_d[t * 128:(t + 1) * 128].unsqueeze(1))
            nc.sync.dma_start(out=bv_sb[:], in_=bv_d[:].unsqueeze(0))

            xq_sb, xk_sb, xv_sb = [], [], []
            for k in range(KT):
                nc.sync.dma_start(out=wk_sb[k][:], in_=wk_d[k * 128:(k + 1) * 128, :])
                xt = xp.tile([128, S], bf16, name=f"xk{k}", tag="xin")
                nc.sync.dma_start(out=xt[:], in_=xk_d[k * 128:(k + 1) * 128, :])
                xk_sb.append(xt)
            for k in range(KT):
                nc.sync.dma_start(out=wq_sb[k][:], in_=wq_d[k * 128:(k + 1) * 128, :])
                xt = xp.tile([128, S], bf16, name=f"xq{k}", tag="xin")
                nc.sync.dma_start(out=xt[:], in_=xq_d[k * 128:(k + 1) * 128, :])
                xq_sb.append(xt)
            for k in range(KT):
                nc.sync.dma_start(out=wv_sb[k][:], in_=wv_d[k * 128:(k + 1) * 128, :])
                xt = xp.tile([128, S], bf16, name=f"xv{k}", tag="xin")
                nc.sync.dma_start(out=xt[:], in_=xv_d[k * 128:(k + 1) * 128, :])
                xv_sb.append(xt)
            for t in range(FT):
                nc.sync.dma_start(out=wo_sb[t][:], in_=wo_d[t * 128:(t + 1) * 128, :])

            with tc.tile_pool(name="psB", bufs=2, space="PSUM") as psB:

                def big(name):
                    return psB.tile([128, S], mybir.dt.float32,
                                    name=name, tag="pssc")

                # q/k projection: both f-tiles accumulate k-outer in two
                # full-S PSUM slots, interleaved per k so each streamed input
                # tile is consumed (and its slot freed) immediately.
                def proj_qk(w_sb, x_sb, b_sb, y_sb):
                    acc0 = big("acc0")
                    acc1 = big("acc1")
                    for k in range(KT):
                        for t, acc in ((0, acc0), (1, acc1)):
                            for n in range(4):
                                nc.tensor.matmul(
                                    acc[:, n * 512:(n + 1) * 512],
                                    lhsT=w_sb[k][:, t * 128:(t + 1) * 128],
                                    rhs=x_sb[k][:, n * 512:(n + 1) * 512],
                                    start=(k == 0), stop=(k == KT - 1))
                    nc.vector.tensor_scalar_add(y_sb[0][:], acc0[:], b_sb[0][:])
                    nc.vector.tensor_scalar_add(y_sb[1][:], acc1[:], b_sb[1][:])

                # v projection for one seq tile (+bias via ones-row matmul)
                def vproj_unit(st):
                    pv = psB.tile([128, F], mybir.dt.float32,
                                  name="pv", tag="pssc")
                    for k in range(KT):
                        nc.tensor.matmul(
                            pv[:], lhsT=xv_sb[k][:, st * 128:(st + 1) * 128],
                            rhs=wv_sb[k][:], start=(k == 0), stop=False)
                    nc.tensor.matmul(pv[:], lhsT=ones_sb[:], rhs=bv_sb[:],
                                     start=False, stop=True)
                    for h in range(GH):
                        nc.vector.tensor_copy(
                            va_sb[st][:, h * (HD + 1):h * (HD + 1) + HD],
                            pv[:, h * HD:(h + 1) * HD])

                def emit_scores(h, j):
                    ht = h // 2
                    off = (h % 2) * HD
                    ps = big("pssc")
                    for n in range(4):
                        nc.tensor.matmul(
                            ps[:, n * 512:(n + 1) * 512],
                            lhsT=kT_sb[ht][off:off + HD, j * 128:(j + 1) * 128],
                            rhs=qT_sb[ht][off:off + HD, n * 512:(n + 1) * 512],
                            start=True, stop=True)
                    e = ep.tile([128, S], bf16, name="expT", tag="expT")
                    nc.scalar.activation(e[:], ps[:], Exp,
                                         bias=mk_sb[j][:], scale=SCALE)
                    return e

                def transpose_o(it):
                    for t in range(FT):
                        pt = psB.tile([128, 128], bf16, name="pst", tag="pssc")
                        nc.tensor.transpose(
                            pt[:], os_sb[it][:, t * 128:(t + 1) * 128], ident[:])
                        nc.vector.tensor_copy(
                            ot_sb[t][:, it * 128:(it + 1) * 128], pt[:])

                def out_proj(ih):
                    i0 = ih * 1024
                    for do in range(DT):
                        pso = psB.tile([128, 1024], mybir.dt.float32,
                                       name="pso", tag="pssc")
                        for n in range(2):
                            for t in range(FT):
                                nc.tensor.matmul(
                                    pso[:, n * 512:(n + 1) * 512],
                                    lhsT=wo_sb[t][:, do * 128:(do + 1) * 128],
                                    rhs=ot_sb[t][:, i0 + n * 512:i0 + (n + 1) * 512],
                                    start=(t == 0), stop=(t == FT - 1))
                        stg = osp.tile([128, 1024], mybir.dt.float32,
                                       name="stg", tag="stg")
                        if do % 2 == 0:
                            nc.vector.tensor_scalar_add(stg[:], pso[:], bo_sb[do][:])
                        else:
                            nc.scalar.add(stg[:], pso[:], bo_sb[do][:])
                        nc.sync.dma_start(
                            out=out_d[do * 128:(do + 1) * 128, i0:i0 + 1024],
                            in_=stg[:])

                # one AV accumulator quad: 4 i-tiles packed into one PSUM slot
                def av_quad(h, ets, quad, tail):
                    pot = big("pot")
                    for sub in range(4):
                        it = quad * 4 + sub
                        po = pot[:, sub * 512:sub * 512 + HD + 1]
                        for j in range(ST):
                            nc.tensor.matmul(
                                po,
                                lhsT=ets[j][:, it * 128:(it + 1) * 128],
                                rhs=va_sb[j][:, h * (HD + 1):(h + 1) * (HD + 1)],
                                start=(j == 0), stop=(j == ST - 1))
                        rec = sp.tile([128, 1], mybir.dt.float32,
                                      name="rec", tag="rec")
                        nc.vector.reciprocal(rec[:], po[:, HD:HD + 1])
                        nc.vector.tensor_scalar_mul(
                            os_sb[it][:, h * HD:(h + 1) * HD],
                            po[:, 0:HD], rec[:])
                        if tail:
                            transpose_o(it)
                    if tail and quad == 1:
                        out_proj(0)
                    elif tail and quad == 3:
                        out_proj(1)

                # ---------------- emission schedule ----------------
                proj_qk(wk_sb, xk_sb, bk_sb, kT_sb)
                proj_qk(wq_sb, xq_sb, bq_sb, qT_sb)

                # extras interleaved between score tiles, one per slot:
                #  head 0: f1 projections then the 16 v-projection units
                #  head h>=1: the 4 AV quads of head h-1 (early, so expT slots
                #  recycle before ScalarE needs them)
                prev = None
                for h in range(GH):
                    sched = {j: [] for j in range(2, ST)}
                    if h == 0:
                        # the 16 v-projection units, 2 per step once xv landed
                        for st in range(ST):
                            sched[6 + st // 2].append(lambda st=st: vproj_unit(st))
                    else:
                        # previous head's AV quads, as early as possible so the
                        # expT slots recycle before ScalarE needs them
                        for q in range(4):
                            sched[2 + q].append(
                                lambda q=q, hh=h - 1, ee=prev:
                                av_quad(hh, ee, q, False))
                    ets = [emit_scores(h, 0), emit_scores(h, 1)]
                    for j in range(2, ST):
                        ets.append(emit_scores(h, j))
                        for fn in sched[j]:
                            fn()
                    prev = ets
                for q in range(4):
                    av_quad(GH - 1, prev, q, True)

    nc.compile()
    return nc


def kernel(query, key, value, src_mask, Wq, bq, Wk, bk, Wv, bv, Wo, bo, nhead):
    global LAST_EXEC_NS, LAST_RESULTS
    import ml_dtypes
    from concourse.bass_utils import run_bass_kernel_spmd

    assert int(nhead) == H
    bf16 = ml_dtypes.bfloat16
    query = np.asarray(query, dtype=np.float32)
    key = np.asarray(key, dtype=np.float32)
    value = np.asarray(value, dtype=np.float32)
    src_mask = np.asarray(src_mask)
    Wq, bq = np.asarray(Wq, np.float32), np.asarray(bq, np.float32)
    Wk, bk = np.asarray(Wk, np.float32), np.asarray(bk, np.float32)
    Wv, bv = np.asarray(Wv, np.float32), np.asarray(bv, np.float32)
    Wo, bo = np.asarray(Wo, np.float32), np.asarray(bo, np.float32)

    if "nc" not in _STATE:
        _STATE["nc"] = _build()
    nc = _STATE["nc"]

    xqT = [np.ascontiguousarray(query[b].T).astype(bf16) for b in range(B)]
    xkT = [np.ascontiguousarray(key[b].T).astype(bf16) for b in range(B)]
    xvT = [np.ascontiguousarray(value[b].T).astype(bf16) for b in range(B)]
    maskf = [np.where(src_mask[b], NEG, np.float32(0)).astype(np.float32)
             for b in range(B)]

    wqT, wkT, wvT, woT, bqs, bks, bvs = [], [], [], [], [], [], []
    for g in range(NCORES // B):
        gs, ge = g * F, (g + 1) * F
        wqT.append(np.ascontiguousarray(Wq[gs:ge, :].T).astype(bf16))
        wkT.append(np.ascontiguousarray(Wk[gs:ge, :].T).astype(bf16))
        wvT.append(np.ascontiguousarray(Wv[gs:ge, :].T).astype(bf16))
        woT.append(np.ascontiguousarray(Wo[:, gs:ge].T).astype(bf16))
        bqs.append(np.ascontiguousarray(bq[gs:ge]))
        bks.append(np.ascontiguousarray(bk[gs:ge]))
        bvs.append(bv[gs:ge].astype(bf16))
    bo_zero = np.zeros_like(bo)

    in_maps = []
    for c in range(NCORES):
        b, g = c // (NCORES // B), c % (NCORES // B)
        in_maps.append({
            "xqT": xqT[b], "xkT": xkT[b], "xvT": xvT[b],
            "wqT": wqT[g], "wkT": wkT[g], "wvT": wvT[g], "woT": woT[g],
            "bq": bqs[g], "bk": bks[g], "bv": bvs[g],
            "bo": bo if g == 0 else bo_zero,
            "maskf": maskf[b],
        })

    kwargs = {}
    if TRACE:
        kwargs = dict(trace=True)
    res = run_bass_kernel_spmd(nc, in_maps, core_ids=list(range(NCORES)),
                               **kwargs)
    LAST_EXEC_NS = res.exec_time_ns
    LAST_RESULTS = res

    out = np.empty((B, S, D), dtype=np.float32)
    for b in range(B):
        acc = res.results[b * (NCORES // B)]["outT"].astype(np.float32)
        for g in range(1, NCORES // B):
            acc = acc + res.results[b * (NCORES // B) + g]["outT"]
        out[b] = acc.T
    return out
